# revision 38
# baseline (speedup 1.0000x reference)
"""Trainium2 Bass kernel for nn_JointModalityAttention.

3-modality joint attention, B=8, N=512, D=512, 8 heads x 64.
Sharding: data-parallel over batch -- each of the 8 NeuronCores handles one
batch element; the weights are replicated. No collectives.

Dataflow (per core, fully transpose-free on device):
  - Host passes x transposed (xT[k, n]) and a q-projection copy with masked
    query rows compacted out (xqT), both bf16 (as are the Wqkv weights) to
    halve input-DMA bytes.  Zeroed q rows -> dots == 0 -> exp(0) == 1 ->
    uniform attention, which is exactly what the reference's whole-row -1e9
    masking produces, so one zero-padding row serves every masked query.
  - Projections run in bf16 and write bf16 SBUF operand tiles: bf16 weights
    halve the per-matmul LDWEIGHTS time, which otherwise serializes between
    matmuls and dominates small-N attention matmuls.
  - The three modalities' queries are packed side by side (W = 3*n_c wide) so
    one dots matmul serves all three: dots^T[key_chunk, q] = kh.T @ qh_all,
    split into NS bank-aligned pieces of WS columns.
  - exp on the Scalar engine straight out of PSUM with the 1/sqrt(dh) scale
    folded in (no max-subtraction: dots are O(1)).  One exp instruction per
    (head, key-chunk) covers all three modalities; the Scalar engine is the
    pacing engine, so projections are emitted per-fh-block as PE filler
    exactly where the attention stream needs them, keeping the exp stream
    dense from ~15us on.
  - attn@V shares the V tile as lhsT across modalities: o^T[d, q_all] accum
    over 12 key chunks; a ones column in V makes row 64 the softmax
    denominator for every query.
  - normalize: copy the denominator row to partition 0 (custom-DVE ops
    ignore input partition offsets), reciprocal_approx_fast, gpsimd
    partition-broadcast, one DVE multiply into the head-pair tile that is
    exactly the lhsT layout the out-projection needs.
  - Emission is software-pipelined: dots runs 2 key-chunks ahead of attn@V
    (PSUM double-buffering), heads are processed fh-major so each fh block's
    k/q projections are produced one pair ahead of use.
"""

import sys

if "/opt/trn_rl_repo" not in sys.path:
    sys.path.insert(0, "/opt/trn_rl_repo")

import ml_dtypes
import numpy as np

import concourse.bass as bass  # noqa: F401  (import keeps bass registered)
import concourse.mybir as mybir
import concourse.tile as tile
from concourse import bacc, bass_utils

HEADS = 8
DH = 64
DI = HEADS * DH  # 512
B = 8
N = 512
D = 512
SCALE = DH ** -0.5
N_CORES = 8

F32 = mybir.dt.float32
BF16 = mybir.dt.bfloat16
F32R = mybir.dt.float32r


def tf32_round(a):
    """Round fp32 to the TF32-representable subset (10-bit mantissa, RNE)."""
    u = np.ascontiguousarray(a, np.float32).view(np.uint32).copy()
    lsb = (u >> 13) & 1
    u += 0x0FFF + lsb
    u &= 0xFFFFE000
    return u.view(np.float32)


def _splits(n_c):
    """(NS, WS): number of bank-aligned pieces the merged q width 3*n_c is
    split into, and the piece width (<=512 per PSUM bank / moving max)."""
    W = 3 * n_c
    ns = 2 if W <= 1024 else 3
    assert W % ns == 0 and W // ns <= 512
    return ns, W // ns


def _emit_body(nc, tc, dio, pools, n_c):
    f32 = F32
    Exp = mybir.ActivationFunctionType.Exp
    (p_wq, p_wkv, p_xt, p_xqt, p_qkv, p_ex, p_den, p_pair, p_wo, p_ob,
     psD, psO) = pools
    NS, WS = _splits(n_c)
    W = 3 * n_c
    PSW = NS * 512
    ps_bufs = 2 if NS == 2 else 1

    def mm(out, lhsT, rhs, start, stop):
        return nc.tensor.matmul(out, lhsT, rhs, start=start, stop=stop)

    ones8 = p_den.tile([128, 8], f32, tag="ones8", name="ones8")
    nc.vector.memset(ones8[:, :], 1.0)

    # ---- input DMA: one instruction per tile.  The SP sequencer spends
    # ~6.5us on its start-of-kernel preamble before it can issue anything,
    # so the startup-critical transfers go out on the Activation and DVE
    # sequencers, which are ready at ~0.2us (both idle until the exp stream
    # / first copies anyway).
    wq, wkv, xt, xqt, wo = {}, {}, {}, {}, {}
    for i in range(3):
        wq[i] = p_wq.tile([128, 4 * 512], BF16, tag="wq", name=f"wq{i}")
        xqt[i] = p_xqt.tile([128, 4 * n_c], BF16, tag="xqt", name=f"xqt{i}")
        nc.scalar.dma_start(
            wq[i][:, :].rearrange("p (kt x) -> p kt x", x=512),
            dio[f"Wqkv{i}"].rearrange("(kt p) x -> p kt x", p=128)[:, :, 0:512],
        )
        nc.scalar.dma_start(
            xqt[i][:, :].rearrange("p (kt n) -> p kt n", n=n_c),
            dio[f"xqT{i}"].rearrange("(kt p) n -> p kt n", p=128),
        )
    for i in range(3):
        wkv[i] = p_wkv.tile([128, 4 * 1024], BF16, tag="wkv", name=f"wkv{i}")
        xt[i] = p_xt.tile([128, 4 * 512], BF16, tag="xt", name=f"xt{i}")
        eng = nc.gpsimd if i == 0 else nc.sync
        eng.dma_start(
            xt[i][:, :].rearrange("p (kt n) -> p kt n", n=512),
            dio[f"xT{i}"].rearrange("(kt p) n -> p kt n", p=128),
        )
        eng.dma_start(
            wkv[i][:, :].rearrange("p (kt x) -> p kt x", x=1024),
            dio[f"Wqkv{i}"].rearrange("(kt p) x -> p kt x", p=128)[:, :, 512:1536],
        )
    for i in range(3):
        wo[i] = p_wo.tile([128, 4 * 512], BF16, tag="wo", name=f"wo{i}")
        nc.sync.dma_start(
            wo[i][:, :].rearrange("p (kt x) -> p kt x", x=512),
            dio[f"Wout{i}"].rearrange("(kt p) x -> p kt x", p=128),
        )

    # ---- PE p-state warmup: zero matmuls with no DMA dependency keep the
    # PE ramping toward full clock while the first weight tiles land
    warm = p_den.tile([128, 512], BF16, tag="warm", name="warm")
    nc.vector.memzero(warm[:, :])
    for wi in range(10):
        psw = psD.tile([128, PSW], f32, tag="d", name=f"warm{wi}")
        mm(psw[:, 0:512], warm[:, 0:128], warm[:, 0:512], True, True)

    # ---- persistent SBUF layouts ----
    # qT_all[128, 4*W]: block fh holds dims of head-pair (2fh, 2fh+1); within
    # a block the three modalities' queries sit side by side (i*n_c + q).
    qT_all = p_qkv.tile([128, 4 * W], BF16, tag="qT", name="qT_all")
    # kT_all[128, 4*1536]: block fh, then modality j * 512 + key.
    kT_all = p_qkv.tile([128, 4 * 1536], BF16, tag="kT", name="kT_all")
    # V_all[128, 12*520]: chunk ci = j*4+c of 128 keys; within: 8 heads x
    # (64 V-dims + ones column).
    V_all = p_qkv.tile([128, 12 * 520], BF16, tag="V", name="V_all")

    def proj_q(i, cc):
        ps = psD.tile([128, PSW], f32, tag="d", name=f"psq{i}_{cc}")
        for kt in range(4):
            mm(
                ps[:, 0:n_c],
                wq[i][:, kt * 512 + cc * 128 : kt * 512 + cc * 128 + 128],
                xqt[i][:, kt * n_c : (kt + 1) * n_c],
                kt == 0,
                kt == 3,
            )
        nc.vector.tensor_copy(
            qT_all[:, cc * W + i * n_c : cc * W + (i + 1) * n_c],
            ps[:, 0:n_c],
        )

    def proj_k(i, cc):
        ps = psD.tile([128, PSW], f32, tag="d", name=f"psk{i}_{cc}")
        for kt in range(4):
            mm(
                ps[:, 0:512],
                wkv[i][:, kt * 1024 + cc * 128 : kt * 1024 + cc * 128 + 128],
                xt[i][:, kt * 512 : (kt + 1) * 512],
                kt == 0,
                kt == 3,
            )
        nc.vector.tensor_copy(
            kT_all[:, cc * 1536 + i * 512 : cc * 1536 + (i + 1) * 512],
            ps[:, 0:512],
        )

    def proj_v(i, nch):
        ps = psD.tile([128, PSW], f32, tag="d", name=f"psv{i}_{nch}")
        for kt in range(4):
            mm(
                ps[:, 0:512],
                xt[i][:, kt * 512 + nch * 128 : kt * 512 + nch * 128 + 128],
                wkv[i][:, kt * 1024 + 512 : kt * 1024 + 1024],
                kt == 0,
                kt == 3,
            )
        ci = i * 4 + nch
        dst = V_all[:, ci * 520 : (ci + 1) * 520].rearrange(
            "p (h x) -> p h x", x=65
        )
        nc.vector.tensor_copy(
            dst[:, :, 0:64], ps[:, 0:512].rearrange("p (h x) -> p h x", x=64)
        )
        nc.vector.tensor_copy(
            dst[:, :, 64:65], ones8[:, :].rearrange("p (h x) -> p h x", x=1)
        )

    # ---- attention stream (software-pipelined, with proj fillers) ----
    AV_LAG = 2
    # Emission points (h, kc) -> projection units emitted just before that
    # dots call.  v(j, c) must precede av(h0, j*4+c) (emitted at kc j*4+c+2);
    # k(j, cc) before the first dots of fh block cc on modality j; q(*, cc)
    # before head pair cc starts.
    fillers = {
        (0, 1): [(proj_v, 0, 0)],
        (0, 2): [(proj_v, 0, 1)],
        (0, 3): [(proj_v, 0, 2), (proj_k, 1, 0)],
        (0, 4): [(proj_v, 0, 3)],
        (0, 5): [(proj_v, 1, 0)],
        (0, 6): [(proj_v, 1, 1), (proj_k, 2, 0)],
        (0, 7): [(proj_v, 1, 2)],
        (0, 8): [(proj_v, 1, 3)],
        (0, 9): [(proj_v, 2, 0)],
        (0, 10): [(proj_v, 2, 1)],
        (0, 11): [(proj_v, 2, 2)],
        (1, 0): [(proj_v, 2, 3)],
        (1, 2): [(proj_q, 0, 1)],
        (1, 4): [(proj_q, 1, 1)],
        (1, 6): [(proj_q, 2, 1)],
        (1, 8): [(proj_k, 0, 1)],
        (1, 10): [(proj_k, 1, 1)],
        (2, 0): [(proj_k, 2, 1)],
        (3, 0): [(proj_q, 0, 2)],
        (3, 2): [(proj_q, 1, 2)],
        (3, 4): [(proj_q, 2, 2)],
        (3, 6): [(proj_k, 0, 2)],
        (3, 8): [(proj_k, 1, 2)],
        (3, 10): [(proj_k, 2, 2)],
        (5, 0): [(proj_q, 0, 3)],
        (5, 2): [(proj_q, 1, 3)],
        (5, 4): [(proj_q, 2, 3)],
        (5, 6): [(proj_k, 0, 3)],
        (5, 8): [(proj_k, 1, 3)],
        (5, 10): [(proj_k, 2, 3)],
    }

    ps_d_t, ex_t, ps_o_t = {}, {}, {}

    def dots(h, kc):
        fh, po = h // 2, (h % 2) * 64
        j, c = kc // 4, kc % 4
        ps_d = psD.tile([128, PSW], f32, tag="d", name=f"psd{h}_{kc}")
        ps_d_t[(h, kc)] = ps_d
        base = fh * 1536 + j * 512 + c * 128
        kh = kT_all[po : po + 64, base : base + 128]
        for s in range(NS):
            mm(
                ps_d[:, s * 512 : s * 512 + WS],
                kh,
                qT_all[po : po + 64, fh * W + s * WS : fh * W + (s + 1) * WS],
                True,
                True,
            )
        ex = p_ex.tile([128, W], BF16, tag="ex", name=f"ex{h}_{kc}")
        ex_t[(h, kc)] = ex
        nc.scalar.activation(
            ex[:, :].rearrange("p (s x) -> p s x", x=WS),
            ps_d[:, :].rearrange("p (s x) -> p s x", x=512)[:, :, 0:WS],
            Exp,
            scale=SCALE,
        )

    def av(h, kc):
        if kc == 0:
            ps_o_t[h] = psO.tile([128, PSW], f32, tag="o", name=f"pso{h}")
        ps_o = ps_o_t[h]
        ci = kc  # chunk index == kc (j*4+c ordering matches V_all layout)
        vh = V_all[:, ci * 520 + h * 65 : ci * 520 + h * 65 + 65]
        ex = ex_t.pop((h, kc))
        for s in range(NS):
            mm(
                ps_o[0:65, s * 512 : s * 512 + WS],
                vh,
                ex[:, s * WS : (s + 1) * WS],
                kc == 0,
                kc == 11,
            )

    def norm(h):
        ps_o = ps_o_t.pop(h)
        # the custom-DVE reciprocal ignores the input partition offset, so
        # first move the denominator row (PSUM partition 64) to an SBUF
        # tile at partition 0.  The chain runs per bank-aligned piece so the
        # two halves pipeline (shorter pair-ready latency for the final head,
        # which gates the out-projection).
        den = p_den.tile([1, PSW], f32, tag="den", name=f"dn{h}")
        rden = p_den.tile([1, PSW], f32, tag="r", name=f"rd{h}")
        bc = p_den.tile([64, PSW], f32, tag="b", name=f"bc{h}")
        po2 = (h % 2) * 64
        dst = pair[h // 2][po2 : po2 + 64, :]
        for s in range(NS):
            sl = slice(s * 512, s * 512 + WS)
            nc.vector.tensor_copy(den[0:1, sl], ps_o[64:65, sl])
            nc.vector.reciprocal_approx_fast(
                out=rden[0:1, sl], in_=den[0:1, sl]
            )
            nc.gpsimd.partition_broadcast(bc[:, sl], rden[0:1, sl])
            nc.vector.tensor_mul(
                dst[:, s * WS : (s + 1) * WS],
                ps_o[0:64, sl],
                bc[:, sl],
            )

    pair = [
        p_pair.tile([128, W], BF16, tag=f"pair{p}", name=f"pair{p}")
        for p in range(4)
    ]

    for i in range(3):
        proj_q(i, 0)
    proj_k(0, 0)

    for h in range(HEADS):
        for kc in range(12):
            for fn, i, cc in fillers.get((h, kc), ()):
                fn(i, cc)
            dots(h, kc)
            if kc == 1 and h > 0:
                av(h - 1, 10)
                av(h - 1, 11)
                norm(h - 1)
            if kc >= AV_LAG:
                av(h, kc - AV_LAG)

    # ---- out-projection, two-phase, overlapped with the attention tail ----
    # Phase A (pair0-2 contributions) only needs heads 0-5, so it runs on
    # the PE while the Scalar engine finishes head 7's exps and the
    # DVE/Pool engines run the last norm chains; phase B adds pair3,
    # copies out (alternating Scalar/Vector, both idle by then), and DMAs.
    # Five packed claims (the two 128-row chunks of one modality per claim;
    # the leftover chunks of different modalities share a claim) keep the
    # claim count close to the 4 available PSUM ring slots.
    groups = []  # each: list of (i, s, nn, bank)
    tail_grp = []
    for i in range(3):
        chunks = [(s, min(128, n_c - s)) for s in range(0, n_c, 128)]
        for g in range(0, len(chunks) - len(chunks) % 2, 2):
            groups.append([(i, s, nn, gi) for gi, (s, nn) in
                           enumerate(chunks[g : g + 2])])
        if len(chunks) % 2:
            s, nn = chunks[-1]
            tail_grp.append((i, s, nn))
    for t in range(0, len(tail_grp), NS):
        groups.append([(i, s, nn, gi % NS) for gi, (i, s, nn) in
                       enumerate(tail_grp[t : t + NS])])
    if ps_bufs == 1:
        # single-buffered PSUM rings (NS == 3 fallback): the phase-A
        # interleave below would deadlock the in-order PE queue, so finish
        # the last head before any out-projection claim
        av(HEADS - 1, 10)
        av(HEADS - 1, 11)
        norm(HEADS - 1)
    claims = []
    for gidx, grp in enumerate(groups):
        pool = psD if gidx % 2 == 0 else psO
        ps = pool.tile([128, PSW], f32, tag="d" if pool is psD else "o",
                       name=f"psf{gidx}")
        claims.append(ps)
        for i, s, nn, bank in grp:
            for p in range(3):
                mm(
                    ps[0:nn, bank * 512 : bank * 512 + 512],
                    pair[p][:, i * n_c + s : i * n_c + s + nn],
                    wo[i][:, p * 512 : (p + 1) * 512],
                    p == 0,
                    False,
                )
        if gidx == 1 and ps_bufs == 2:
            av(HEADS - 1, 10)
            av(HEADS - 1, 11)
            norm(HEADS - 1)
    for gidx, grp in enumerate(groups):
        ps = claims[gidx]
        for i, s, nn, bank in grp:
            mm(
                ps[0:nn, bank * 512 : bank * 512 + 512],
                pair[3][:, i * n_c + s : i * n_c + s + nn],
                wo[i][:, 3 * 512 : 4 * 512],
                False,
                True,
            )
        ob = p_ob.tile([128, PSW], f32, tag="ob", name=f"ob{gidx}")
        for k, (i, s, nn, bank) in enumerate(grp):
            cp = nc.scalar.copy if (gidx + k) % 2 == 0 else nc.vector.tensor_copy
            cp(
                ob[0:nn, bank * 512 : bank * 512 + 512],
                ps[0:nn, bank * 512 : bank * 512 + 512],
            )
            nc.sync.dma_start(
                dio[f"out{i}"][s : s + nn, :],
                ob[0:nn, bank * 512 : bank * 512 + 512],
            )


def build(n_c=280, reps=1):
    nc = bacc.Bacc("TRN2", target_bir_lowering=False, debug=False)
    NS, _ = _splits(n_c)
    ps_bufs = 2 if NS == 2 else 1
    dio = {}
    for i in range(3):
        dio[f"xT{i}"] = nc.dram_tensor(f"xT{i}", [D, N], BF16, kind="ExternalInput").ap()
        dio[f"xqT{i}"] = nc.dram_tensor(
            f"xqT{i}", [D, n_c], BF16, kind="ExternalInput"
        ).ap()
        dio[f"Wqkv{i}"] = nc.dram_tensor(
            f"Wqkv{i}", [D, 3 * DI], BF16, kind="ExternalInput"
        ).ap()
        dio[f"Wout{i}"] = nc.dram_tensor(
            f"Wout{i}", [DI, D], BF16, kind="ExternalInput"
        ).ap()
        dio[f"out{i}"] = nc.dram_tensor(
            f"out{i}", [n_c, D], F32, kind="ExternalOutput"
        ).ap()
    with tile.TileContext(nc) as tc:
        with (
            tc.tile_pool(name="wq", bufs=3) as p_wq,
            tc.tile_pool(name="wkv", bufs=3) as p_wkv,
            tc.tile_pool(name="xt", bufs=3) as p_xt,
            tc.tile_pool(name="xqt", bufs=3) as p_xqt,
            tc.tile_pool(name="qkv", bufs=1) as p_qkv,
            tc.tile_pool(name="ex", bufs=4) as p_ex,
            tc.tile_pool(name="den", bufs=2) as p_den,
            tc.tile_pool(name="pair", bufs=1) as p_pair,
            tc.tile_pool(name="wo", bufs=3) as p_wo,
            tc.tile_pool(name="ob", bufs=2) as p_ob,
            tc.tile_pool(name="psD", bufs=ps_bufs, space="PSUM") as psD,
            tc.tile_pool(name="psO", bufs=ps_bufs, space="PSUM") as psO,
        ):
            pools = (p_wq, p_wkv, p_xt, p_xqt, p_qkv, p_ex, p_den, p_pair,
                     p_wo, p_ob, psD, psO)
            for _ in range(reps):
                _emit_body(nc, tc, dio, pools, n_c)
    nc.compile()
    return nc


_BUILD_CACHE = {}


def _get_built(n_c, reps):
    key = (n_c, reps)
    if key not in _BUILD_CACHE:
        _BUILD_CACHE[key] = build(n_c, reps)
    return _BUILD_CACHE[key]


def pick_n_c(inputs):
    """Smallest supported compacted-query count for these masks.

    Per (b, i) we need room for the unmasked queries plus one zero (dummy)
    row whose output serves every masked query of that (b, i)."""
    need = 0
    for i in range(3):
        m = np.asarray(inputs[f"m{i}"]).astype(bool)
        for b in range(B):
            n_u = int(m[b].sum())
            need = max(need, n_u + (1 if n_u < N else 0))
    n_c = max(256, -(-need // 8) * 8)
    if n_c > 336:  # NS=2 supports 3*n_c/2 <= 512
        for cand in (384, 448, 512):
            if need <= cand:
                return cand
        return 512
    return n_c


def make_in_maps(inputs, n_c=280):
    """Build per-core input dicts.  The q-projection input is compacted to
    the unmasked query rows (plus zero padding; the first padding row doubles
    as the masked-row output).  x / xq / Wqkv / Wout ship as bf16."""
    bf = ml_dtypes.bfloat16
    xs = [np.asarray(inputs[f"x{i}"], dtype=np.float32) for i in range(3)]
    ms = [np.asarray(inputs[f"m{i}"]).astype(bool) for i in range(3)]
    Wq = [np.asarray(inputs[f"Wqkv{i}"], np.float32).astype(bf) for i in range(3)]
    Wo = [np.asarray(inputs[f"Wout{i}"], np.float32).astype(bf) for i in range(3)]
    in_maps = []
    for b in range(B):
        m = {}
        for i in range(3):
            xb = xs[i][b]
            m[f"xT{i}"] = np.ascontiguousarray(xb.T).astype(bf)
            if n_c == N:
                xq = xb * ms[i][b][:, None]
            else:
                sel = np.flatnonzero(ms[i][b])
                xq = np.zeros((n_c, D), np.float32)
                xq[: len(sel)] = xb[sel]
            m[f"xqT{i}"] = np.ascontiguousarray(xq.T).astype(bf)
            m[f"Wqkv{i}"] = Wq[i]
            m[f"Wout{i}"] = Wo[i]
        in_maps.append(m)
    return in_maps


def scatter_outputs(results, inputs, n_c):
    ms = [np.asarray(inputs[f"m{i}"]).astype(bool) for i in range(3)]
    outs = []
    for i in range(3):
        full = np.empty((B, N, D), np.float32)
        for b in range(B):
            comp = np.asarray(results[b][f"out{i}"], np.float32)
            if n_c == N:
                full[b] = comp
            else:
                sel = np.flatnonzero(ms[i][b])
                full[b][sel] = comp[: len(sel)]
                if len(sel) < N:
                    full[b][~ms[i][b]] = comp[len(sel)]
        outs.append(full)
    return outs


def kernel(**inputs):
    n_c = pick_n_c(inputs)
    in_maps = make_in_maps(inputs, n_c)
    nc = _get_built(n_c=n_c, reps=1)
    res = bass_utils.run_bass_kernel_spmd(nc, in_maps, core_ids=list(range(N_CORES)))
    return tuple(scatter_outputs(res.results, inputs, n_c))


# revision 39
# speedup vs baseline: 1.0642x; 1.0642x over previous
"""Trainium2 Bass kernel for nn_JointModalityAttention.

3-modality joint attention, B=8, N=512, D=512, 8 heads x 64.
Sharding: data-parallel over batch -- each of the 8 NeuronCores handles one
batch element; the weights are replicated. No collectives.

Dataflow (per core, fully transpose-free on device):
  - Host passes x transposed (xT[k, n]) and a q-projection copy with masked
    query rows compacted out (xqT), both bf16 (as are the Wqkv weights) to
    halve input-DMA bytes.  Zeroed q rows -> dots == 0 -> exp(0) == 1 ->
    uniform attention, which is exactly what the reference's whole-row -1e9
    masking produces, so one zero-padding row serves every masked query.
  - Projections run in bf16 and write bf16 SBUF operand tiles: bf16 weights
    halve the per-matmul LDWEIGHTS time, which otherwise serializes between
    matmuls and dominates small-N attention matmuls.
  - The three modalities' queries are packed side by side (W = 3*n_c wide) so
    one dots matmul serves all three: dots^T[key_chunk, q] = kh.T @ qh_all,
    split into NS bank-aligned pieces of WS columns.
  - exp on the Scalar engine straight out of PSUM with the 1/sqrt(dh) scale
    folded in (no max-subtraction: dots are O(1)).  One exp instruction per
    (head, key-chunk) covers all three modalities; the Scalar engine is the
    pacing engine, so projections are emitted per-fh-block as PE filler
    exactly where the attention stream needs them, keeping the exp stream
    dense from ~15us on.
  - attn@V shares the V tile as lhsT across modalities: o^T[d, q_all] accum
    over 12 key chunks; a ones column in V makes row 64 the softmax
    denominator for every query.
  - normalize: copy the denominator row to partition 0 (custom-DVE ops
    ignore input partition offsets), reciprocal_approx_fast, gpsimd
    partition-broadcast, one DVE multiply into the head-pair tile that is
    exactly the lhsT layout the out-projection needs.
  - Emission is software-pipelined: dots runs 2 key-chunks ahead of attn@V
    (PSUM double-buffering), heads are processed fh-major so each fh block's
    k/q projections are produced one pair ahead of use.
"""

import sys

if "/opt/trn_rl_repo" not in sys.path:
    sys.path.insert(0, "/opt/trn_rl_repo")

import ml_dtypes
import numpy as np

import concourse.bass as bass  # noqa: F401  (import keeps bass registered)
import concourse.mybir as mybir
import concourse.tile as tile
from concourse import bacc, bass_utils

HEADS = 8
DH = 64
DI = HEADS * DH  # 512
B = 8
N = 512
D = 512
SCALE = DH ** -0.5
N_CORES = 8

F32 = mybir.dt.float32
BF16 = mybir.dt.bfloat16
F32R = mybir.dt.float32r


def tf32_round(a):
    """Round fp32 to the TF32-representable subset (10-bit mantissa, RNE)."""
    u = np.ascontiguousarray(a, np.float32).view(np.uint32).copy()
    lsb = (u >> 13) & 1
    u += 0x0FFF + lsb
    u &= 0xFFFFE000
    return u.view(np.float32)


def _splits(n_c):
    """(NS, WS): number of bank-aligned pieces the merged q width 3*n_c is
    split into, and the piece width (<=512 per PSUM bank / moving max)."""
    W = 3 * n_c
    ns = 2 if W <= 1024 else 3
    assert W % ns == 0 and W // ns <= 512
    return ns, W // ns


def _emit_body(nc, tc, dio, pools, n_c):
    f32 = F32
    Exp = mybir.ActivationFunctionType.Exp
    (p_wq, p_wkv, p_xt, p_xqt, p_qkv, p_ex, p_den, p_pair, p_wo, p_ob,
     psD, psO) = pools
    NS, WS = _splits(n_c)
    W = 3 * n_c
    PSW = NS * 512
    ps_bufs = 2 if NS == 2 else 1

    def mm(out, lhsT, rhs, start, stop):
        return nc.tensor.matmul(out, lhsT, rhs, start=start, stop=stop)

    ones8 = p_den.tile([128, 8], f32, tag="ones8", name="ones8")
    nc.vector.memset(ones8[:, :], 1.0)

    # ---- input DMA: one instruction per tile.  The SP sequencer spends
    # ~6.5us on its start-of-kernel preamble before it can issue anything,
    # so the startup-critical transfers go out on the Activation and DVE
    # sequencers, which are ready at ~0.2us (both idle until the exp stream
    # / first copies anyway).
    wq, wkv, xt, xqt, wo = {}, {}, {}, {}, {}
    for i in range(3):
        wq[i] = p_wq.tile([128, 4 * 512], BF16, tag="wq", name=f"wq{i}")
        xqt[i] = p_xqt.tile([128, 4 * n_c], BF16, tag="xqt", name=f"xqt{i}")
        eng = nc.scalar if i == 0 else nc.sync
        eng.dma_start(
            wq[i][:, :].rearrange("p (kt x) -> p kt x", x=512),
            dio[f"Wqkv{i}"].rearrange("(kt p) x -> p kt x", p=128)[:, :, 0:512],
        )
        eng.dma_start(
            xqt[i][:, :].rearrange("p (kt n) -> p kt n", n=n_c),
            dio[f"xqT{i}"].rearrange("(kt p) n -> p kt n", p=128),
        )
    for i in range(3):
        wkv[i] = p_wkv.tile([128, 4 * 1024], BF16, tag="wkv", name=f"wkv{i}")
        xt[i] = p_xt.tile([128, 4 * 512], BF16, tag="xt", name=f"xt{i}")
        nc.sync.dma_start(
            xt[i][:, :].rearrange("p (kt n) -> p kt n", n=512),
            dio[f"xT{i}"].rearrange("(kt p) n -> p kt n", p=128),
        )
        nc.sync.dma_start(
            wkv[i][:, :].rearrange("p (kt x) -> p kt x", x=1024),
            dio[f"Wqkv{i}"].rearrange("(kt p) x -> p kt x", p=128)[:, :, 512:1536],
        )
    for i in range(3):
        wo[i] = p_wo.tile([128, 4 * 512], BF16, tag="wo", name=f"wo{i}")
        nc.sync.dma_start(
            wo[i][:, :].rearrange("p (kt x) -> p kt x", x=512),
            dio[f"Wout{i}"].rearrange("(kt p) x -> p kt x", p=128),
        )

    # ---- persistent SBUF layouts ----
    # qT_all[128, 4*W]: block fh holds dims of head-pair (2fh, 2fh+1); within
    # a block the three modalities' queries sit side by side (i*n_c + q).
    qT_all = p_qkv.tile([128, 4 * W], BF16, tag="qT", name="qT_all")
    # kT_all[128, 4*1536]: block fh, then modality j * 512 + key.
    kT_all = p_qkv.tile([128, 4 * 1536], BF16, tag="kT", name="kT_all")
    # V_all[128, 12*520]: chunk ci = j*4+c of 128 keys; within: 8 heads x
    # (64 V-dims + ones column).
    V_all = p_qkv.tile([128, 12 * 520], BF16, tag="V", name="V_all")

    def proj_q(i, cc):
        ps = psD.tile([128, PSW], f32, tag="d", name=f"psq{i}_{cc}")
        for kt in range(4):
            mm(
                ps[:, 0:n_c],
                wq[i][:, kt * 512 + cc * 128 : kt * 512 + cc * 128 + 128],
                xqt[i][:, kt * n_c : (kt + 1) * n_c],
                kt == 0,
                kt == 3,
            )
        nc.vector.tensor_copy(
            qT_all[:, cc * W + i * n_c : cc * W + (i + 1) * n_c],
            ps[:, 0:n_c],
        )

    def proj_k(i, cc):
        ps = psD.tile([128, PSW], f32, tag="d", name=f"psk{i}_{cc}")
        for kt in range(4):
            mm(
                ps[:, 0:512],
                wkv[i][:, kt * 1024 + cc * 128 : kt * 1024 + cc * 128 + 128],
                xt[i][:, kt * 512 : (kt + 1) * 512],
                kt == 0,
                kt == 3,
            )
        nc.vector.tensor_copy(
            kT_all[:, cc * 1536 + i * 512 : cc * 1536 + (i + 1) * 512],
            ps[:, 0:512],
        )

    def proj_v(i, nch):
        ps = psD.tile([128, PSW], f32, tag="d", name=f"psv{i}_{nch}")
        for kt in range(4):
            mm(
                ps[:, 0:512],
                xt[i][:, kt * 512 + nch * 128 : kt * 512 + nch * 128 + 128],
                wkv[i][:, kt * 1024 + 512 : kt * 1024 + 1024],
                kt == 0,
                kt == 3,
            )
        ci = i * 4 + nch
        dst = V_all[:, ci * 520 : (ci + 1) * 520].rearrange(
            "p (h x) -> p h x", x=65
        )
        nc.vector.tensor_copy(
            dst[:, :, 0:64], ps[:, 0:512].rearrange("p (h x) -> p h x", x=64)
        )
        nc.vector.tensor_copy(
            dst[:, :, 64:65], ones8[:, :].rearrange("p (h x) -> p h x", x=1)
        )

    # ---- attention stream (software-pipelined, with proj fillers) ----
    AV_LAG = 2
    # Emission points (h, kc) -> projection units emitted just before that
    # dots call.  v(j, c) must precede av(h0, j*4+c) (emitted at kc j*4+c+2);
    # k(j, cc) before the first dots of fh block cc on modality j; q(*, cc)
    # before head pair cc starts.
    fillers = {
        (0, 1): [(proj_v, 0, 0)],
        (0, 2): [(proj_v, 0, 1)],
        (0, 3): [(proj_v, 0, 2), (proj_k, 1, 0)],
        (0, 4): [(proj_v, 0, 3)],
        (0, 5): [(proj_v, 1, 0)],
        (0, 6): [(proj_v, 1, 1), (proj_k, 2, 0)],
        (0, 7): [(proj_v, 1, 2)],
        (0, 8): [(proj_v, 1, 3)],
        (0, 9): [(proj_v, 2, 0)],
        (0, 10): [(proj_v, 2, 1)],
        (0, 11): [(proj_v, 2, 2)],
        (1, 0): [(proj_v, 2, 3)],
        (1, 2): [(proj_q, 0, 1)],
        (1, 4): [(proj_q, 1, 1)],
        (1, 6): [(proj_q, 2, 1)],
        (1, 8): [(proj_k, 0, 1)],
        (1, 10): [(proj_k, 1, 1)],
        (2, 0): [(proj_k, 2, 1)],
        (3, 0): [(proj_q, 0, 2)],
        (3, 2): [(proj_q, 1, 2)],
        (3, 4): [(proj_q, 2, 2)],
        (3, 6): [(proj_k, 0, 2)],
        (3, 8): [(proj_k, 1, 2)],
        (3, 10): [(proj_k, 2, 2)],
        (5, 0): [(proj_q, 0, 3)],
        (5, 2): [(proj_q, 1, 3)],
        (5, 4): [(proj_q, 2, 3)],
        (5, 6): [(proj_k, 0, 3)],
        (5, 8): [(proj_k, 1, 3)],
        (5, 10): [(proj_k, 2, 3)],
    }

    ps_d_t, ex_t, ps_o_t = {}, {}, {}

    def dots(h, kc):
        fh, po = h // 2, (h % 2) * 64
        j, c = kc // 4, kc % 4
        ps_d = psD.tile([128, PSW], f32, tag="d", name=f"psd{h}_{kc}")
        ps_d_t[(h, kc)] = ps_d
        base = fh * 1536 + j * 512 + c * 128
        kh = kT_all[po : po + 64, base : base + 128]
        for s in range(NS):
            mm(
                ps_d[:, s * 512 : s * 512 + WS],
                kh,
                qT_all[po : po + 64, fh * W + s * WS : fh * W + (s + 1) * WS],
                True,
                True,
            )
        ex = p_ex.tile([128, W], BF16, tag="ex", name=f"ex{h}_{kc}")
        ex_t[(h, kc)] = ex
        nc.scalar.activation(
            ex[:, :].rearrange("p (s x) -> p s x", x=WS),
            ps_d[:, :].rearrange("p (s x) -> p s x", x=512)[:, :, 0:WS],
            Exp,
            scale=SCALE,
        )

    def av(h, kc):
        if kc == 0:
            ps_o_t[h] = psO.tile([128, PSW], f32, tag="o", name=f"pso{h}")
        ps_o = ps_o_t[h]
        ci = kc  # chunk index == kc (j*4+c ordering matches V_all layout)
        vh = V_all[:, ci * 520 + h * 65 : ci * 520 + h * 65 + 65]
        ex = ex_t.pop((h, kc))
        for s in range(NS):
            mm(
                ps_o[0:65, s * 512 : s * 512 + WS],
                vh,
                ex[:, s * WS : (s + 1) * WS],
                kc == 0,
                kc == 11,
            )

    def norm(h):
        ps_o = ps_o_t.pop(h)
        # the custom-DVE reciprocal ignores the input partition offset, so
        # first move the denominator row (PSUM partition 64) to an SBUF
        # tile at partition 0.  The chain runs per bank-aligned piece so the
        # two halves pipeline (shorter pair-ready latency for the final head,
        # which gates the out-projection).
        den = p_den.tile([1, PSW], f32, tag="den", name=f"dn{h}")
        rden = p_den.tile([1, PSW], f32, tag="r", name=f"rd{h}")
        bc = p_den.tile([64, PSW], f32, tag="b", name=f"bc{h}")
        po2 = (h % 2) * 64
        dst = pair[h // 2][po2 : po2 + 64, :]
        for s in range(NS):
            sl = slice(s * 512, s * 512 + WS)
            nc.vector.tensor_copy(den[0:1, sl], ps_o[64:65, sl])
            nc.vector.reciprocal_approx_fast(
                out=rden[0:1, sl], in_=den[0:1, sl]
            )
            nc.gpsimd.partition_broadcast(bc[:, sl], rden[0:1, sl])
            nc.vector.tensor_mul(
                dst[:, s * WS : (s + 1) * WS],
                ps_o[0:64, sl],
                bc[:, sl],
            )

    pair = [
        p_pair.tile([128, W], BF16, tag=f"pair{p}", name=f"pair{p}")
        for p in range(4)
    ]

    for i in range(3):
        proj_q(i, 0)
    proj_k(0, 0)

    for h in range(HEADS):
        for kc in range(12):
            for fn, i, cc in fillers.get((h, kc), ()):
                fn(i, cc)
            dots(h, kc)
            if kc == 1 and h > 0:
                av(h - 1, 10)
                av(h - 1, 11)
                norm(h - 1)
            if kc >= AV_LAG:
                av(h, kc - AV_LAG)

    # ---- out-projection, two-phase, overlapped with the attention tail ----
    # Phase A (pair0-2 contributions) only needs heads 0-5, so it runs on
    # the PE while the Scalar engine finishes head 7's exps and the
    # DVE/Pool engines run the last norm chains; phase B adds pair3,
    # copies out (alternating Scalar/Vector, both idle by then), and DMAs.
    # Five packed claims (the two 128-row chunks of one modality per claim;
    # the leftover chunks of different modalities share a claim) keep the
    # claim count close to the 4 available PSUM ring slots.
    groups = []  # each: list of (i, s, nn, bank)
    tail_grp = []
    for i in range(3):
        chunks = [(s, min(128, n_c - s)) for s in range(0, n_c, 128)]
        for g in range(0, len(chunks) - len(chunks) % 2, 2):
            groups.append([(i, s, nn, gi) for gi, (s, nn) in
                           enumerate(chunks[g : g + 2])])
        if len(chunks) % 2:
            s, nn = chunks[-1]
            tail_grp.append((i, s, nn))
    for t in range(0, len(tail_grp), NS):
        groups.append([(i, s, nn, gi % NS) for gi, (i, s, nn) in
                       enumerate(tail_grp[t : t + NS])])
    if ps_bufs == 1:
        # single-buffered PSUM rings (NS == 3 fallback): the phase-A
        # interleave below would deadlock the in-order PE queue, so finish
        # the last head before any out-projection claim
        av(HEADS - 1, 10)
        av(HEADS - 1, 11)
        norm(HEADS - 1)
    claims = []
    for gidx, grp in enumerate(groups):
        pool = psD if gidx % 2 == 0 else psO
        ps = pool.tile([128, PSW], f32, tag="d" if pool is psD else "o",
                       name=f"psf{gidx}")
        claims.append(ps)
        for i, s, nn, bank in grp:
            for p in range(3):
                mm(
                    ps[0:nn, bank * 512 : bank * 512 + 512],
                    pair[p][:, i * n_c + s : i * n_c + s + nn],
                    wo[i][:, p * 512 : (p + 1) * 512],
                    p == 0,
                    False,
                )
        if gidx == 1 and ps_bufs == 2:
            av(HEADS - 1, 10)
            av(HEADS - 1, 11)
            norm(HEADS - 1)
    for gidx, grp in enumerate(groups):
        ps = claims[gidx]
        for i, s, nn, bank in grp:
            mm(
                ps[0:nn, bank * 512 : bank * 512 + 512],
                pair[3][:, i * n_c + s : i * n_c + s + nn],
                wo[i][:, 3 * 512 : 4 * 512],
                False,
                True,
            )
        ob = p_ob.tile([128, PSW], f32, tag="ob", name=f"ob{gidx}")
        for k, (i, s, nn, bank) in enumerate(grp):
            cp = nc.scalar.copy if (gidx + k) % 2 == 0 else nc.vector.tensor_copy
            cp(
                ob[0:nn, bank * 512 : bank * 512 + 512],
                ps[0:nn, bank * 512 : bank * 512 + 512],
            )
            nc.sync.dma_start(
                dio[f"out{i}"][s : s + nn, :],
                ob[0:nn, bank * 512 : bank * 512 + 512],
            )


def build(n_c=280, reps=1):
    nc = bacc.Bacc("TRN2", target_bir_lowering=False, debug=False)
    NS, _ = _splits(n_c)
    ps_bufs = 2 if NS == 2 else 1
    dio = {}
    for i in range(3):
        dio[f"xT{i}"] = nc.dram_tensor(f"xT{i}", [D, N], BF16, kind="ExternalInput").ap()
        dio[f"xqT{i}"] = nc.dram_tensor(
            f"xqT{i}", [D, n_c], BF16, kind="ExternalInput"
        ).ap()
        dio[f"Wqkv{i}"] = nc.dram_tensor(
            f"Wqkv{i}", [D, 3 * DI], BF16, kind="ExternalInput"
        ).ap()
        dio[f"Wout{i}"] = nc.dram_tensor(
            f"Wout{i}", [DI, D], BF16, kind="ExternalInput"
        ).ap()
        dio[f"out{i}"] = nc.dram_tensor(
            f"out{i}", [n_c, D], F32, kind="ExternalOutput"
        ).ap()
    with tile.TileContext(nc) as tc:
        with (
            tc.tile_pool(name="wq", bufs=3) as p_wq,
            tc.tile_pool(name="wkv", bufs=3) as p_wkv,
            tc.tile_pool(name="xt", bufs=3) as p_xt,
            tc.tile_pool(name="xqt", bufs=3) as p_xqt,
            tc.tile_pool(name="qkv", bufs=1) as p_qkv,
            tc.tile_pool(name="ex", bufs=4) as p_ex,
            tc.tile_pool(name="den", bufs=2) as p_den,
            tc.tile_pool(name="pair", bufs=1) as p_pair,
            tc.tile_pool(name="wo", bufs=3) as p_wo,
            tc.tile_pool(name="ob", bufs=2) as p_ob,
            tc.tile_pool(name="psD", bufs=ps_bufs, space="PSUM") as psD,
            tc.tile_pool(name="psO", bufs=ps_bufs, space="PSUM") as psO,
        ):
            pools = (p_wq, p_wkv, p_xt, p_xqt, p_qkv, p_ex, p_den, p_pair,
                     p_wo, p_ob, psD, psO)
            for _ in range(reps):
                _emit_body(nc, tc, dio, pools, n_c)
    nc.compile()
    return nc


_BUILD_CACHE = {}


def _get_built(n_c, reps):
    key = (n_c, reps)
    if key not in _BUILD_CACHE:
        _BUILD_CACHE[key] = build(n_c, reps)
    return _BUILD_CACHE[key]


def pick_n_c(inputs):
    """Smallest supported compacted-query count for these masks.

    Per (b, i) we need room for the unmasked queries plus one zero (dummy)
    row whose output serves every masked query of that (b, i)."""
    need = 0
    for i in range(3):
        m = np.asarray(inputs[f"m{i}"]).astype(bool)
        for b in range(B):
            n_u = int(m[b].sum())
            need = max(need, n_u + (1 if n_u < N else 0))
    n_c = max(256, -(-need // 8) * 8)
    if n_c > 336:  # NS=2 supports 3*n_c/2 <= 512
        for cand in (384, 448, 512):
            if need <= cand:
                return cand
        return 512
    return n_c


def make_in_maps(inputs, n_c=280):
    """Build per-core input dicts.  The q-projection input is compacted to
    the unmasked query rows (plus zero padding; the first padding row doubles
    as the masked-row output).  x / xq / Wqkv / Wout ship as bf16."""
    bf = ml_dtypes.bfloat16
    xs = [np.asarray(inputs[f"x{i}"], dtype=np.float32) for i in range(3)]
    ms = [np.asarray(inputs[f"m{i}"]).astype(bool) for i in range(3)]
    Wq = [np.asarray(inputs[f"Wqkv{i}"], np.float32).astype(bf) for i in range(3)]
    Wo = [np.asarray(inputs[f"Wout{i}"], np.float32).astype(bf) for i in range(3)]
    in_maps = []
    for b in range(B):
        m = {}
        for i in range(3):
            xb = xs[i][b]
            m[f"xT{i}"] = np.ascontiguousarray(xb.T).astype(bf)
            if n_c == N:
                xq = xb * ms[i][b][:, None]
            else:
                sel = np.flatnonzero(ms[i][b])
                xq = np.zeros((n_c, D), np.float32)
                xq[: len(sel)] = xb[sel]
            m[f"xqT{i}"] = np.ascontiguousarray(xq.T).astype(bf)
            m[f"Wqkv{i}"] = Wq[i]
            m[f"Wout{i}"] = Wo[i]
        in_maps.append(m)
    return in_maps


def scatter_outputs(results, inputs, n_c):
    ms = [np.asarray(inputs[f"m{i}"]).astype(bool) for i in range(3)]
    outs = []
    for i in range(3):
        full = np.empty((B, N, D), np.float32)
        for b in range(B):
            comp = np.asarray(results[b][f"out{i}"], np.float32)
            if n_c == N:
                full[b] = comp
            else:
                sel = np.flatnonzero(ms[i][b])
                full[b][sel] = comp[: len(sel)]
                if len(sel) < N:
                    full[b][~ms[i][b]] = comp[len(sel)]
        outs.append(full)
    return outs


def kernel(**inputs):
    n_c = pick_n_c(inputs)
    in_maps = make_in_maps(inputs, n_c)
    nc = _get_built(n_c=n_c, reps=1)
    res = bass_utils.run_bass_kernel_spmd(nc, in_maps, core_ids=list(range(N_CORES)))
    return tuple(scatter_outputs(res.results, inputs, n_c))


# revision 40
# speedup vs baseline: 1.0800x; 1.0149x over previous
"""Trainium2 Bass kernel for nn_JointModalityAttention.

3-modality joint attention, B=8, N=512, D=512, 8 heads x 64.
Sharding: data-parallel over batch -- each of the 8 NeuronCores handles one
batch element; the weights are replicated. No collectives.

Dataflow (per core, fully transpose-free on device):
  - Host passes x transposed (xT[k, n]) and a q-projection copy with masked
    query rows compacted out (xqT), both bf16 (as are the Wqkv weights) to
    halve input-DMA bytes.  Zeroed q rows -> dots == 0 -> exp(0) == 1 ->
    uniform attention, which is exactly what the reference's whole-row -1e9
    masking produces, so one zero-padding row serves every masked query.
  - Projections run in bf16 and write bf16 SBUF operand tiles: bf16 weights
    halve the per-matmul LDWEIGHTS time, which otherwise serializes between
    matmuls and dominates small-N attention matmuls.
  - The three modalities' queries are packed side by side (W = 3*n_c wide) so
    one dots matmul serves all three: dots^T[key_chunk, q] = kh.T @ qh_all,
    split into NS bank-aligned pieces of WS columns.
  - exp on the Scalar engine straight out of PSUM with the 1/sqrt(dh) scale
    folded in (no max-subtraction: dots are O(1)).  One exp instruction per
    (head, key-chunk) covers all three modalities; the Scalar engine is the
    pacing engine, so projections are emitted per-fh-block as PE filler
    exactly where the attention stream needs them, keeping the exp stream
    dense from ~15us on.
  - attn@V shares the V tile as lhsT across modalities: o^T[d, q_all] accum
    over 12 key chunks; a ones column in V makes row 64 the softmax
    denominator for every query.
  - normalize: copy the denominator row to partition 0 (custom-DVE ops
    ignore input partition offsets), reciprocal_approx_fast, gpsimd
    partition-broadcast, one DVE multiply into the head-pair tile that is
    exactly the lhsT layout the out-projection needs.
  - Emission is software-pipelined: dots runs 2 key-chunks ahead of attn@V
    (PSUM double-buffering), heads are processed fh-major so each fh block's
    k/q projections are produced one pair ahead of use.
"""

import sys

if "/opt/trn_rl_repo" not in sys.path:
    sys.path.insert(0, "/opt/trn_rl_repo")

import ml_dtypes
import numpy as np

import concourse.bass as bass  # noqa: F401  (import keeps bass registered)
import concourse.mybir as mybir
import concourse.tile as tile
from concourse import bacc, bass_utils

HEADS = 8
DH = 64
DI = HEADS * DH  # 512
B = 8
N = 512
D = 512
SCALE = DH ** -0.5
N_CORES = 8

F32 = mybir.dt.float32
BF16 = mybir.dt.bfloat16
F32R = mybir.dt.float32r


def tf32_round(a):
    """Round fp32 to the TF32-representable subset (10-bit mantissa, RNE)."""
    u = np.ascontiguousarray(a, np.float32).view(np.uint32).copy()
    lsb = (u >> 13) & 1
    u += 0x0FFF + lsb
    u &= 0xFFFFE000
    return u.view(np.float32)


def _splits(n_c):
    """(NS, WS): number of bank-aligned pieces the merged q width 3*n_c is
    split into, and the piece width (<=512 per PSUM bank / moving max)."""
    W = 3 * n_c
    ns = 2 if W <= 1024 else 3
    assert W % ns == 0 and W // ns <= 512
    return ns, W // ns


def _emit_body(nc, tc, dio, pools, n_c):
    f32 = F32
    Exp = mybir.ActivationFunctionType.Exp
    (p_wq, p_wkv, p_xt, p_xqt, p_qkv, p_ex, p_den, p_pair, p_wo, p_ob,
     psD, psO) = pools
    NS, WS = _splits(n_c)
    W = 3 * n_c
    PSW = NS * 512
    ps_bufs = 2 if NS == 2 else 1

    def mm(out, lhsT, rhs, start, stop):
        return nc.tensor.matmul(out, lhsT, rhs, start=start, stop=stop)

    ones8 = p_den.tile([128, 8], f32, tag="ones8", name="ones8")
    nc.vector.memset(ones8[:, :], 1.0)

    # ---- input DMA: one instruction per tile.  The SP sequencer spends
    # ~6.5us on its start-of-kernel preamble before it can issue anything,
    # so the startup-critical transfers go out on the Activation and DVE
    # sequencers, which are ready at ~0.2us (both idle until the exp stream
    # / first copies anyway).
    wq, wkv, xt, xqt, wo = {}, {}, {}, {}, {}
    for i in range(3):
        wq[i] = p_wq.tile([128, 4 * 512], BF16, tag="wq", name=f"wq{i}")
        xqt[i] = p_xqt.tile([128, 4 * n_c], BF16, tag="xqt", name=f"xqt{i}")
        nc.sync.dma_start(
            wq[i][:, :].rearrange("p (kt x) -> p kt x", x=512),
            dio[f"Wqkv{i}"].rearrange("(kt p) x -> p kt x", p=128)[:, :, 0:512],
        )
        nc.sync.dma_start(
            xqt[i][:, :].rearrange("p (kt n) -> p kt n", n=n_c),
            dio[f"xqT{i}"].rearrange("(kt p) n -> p kt n", p=128),
        )
    for i in range(3):
        wkv[i] = p_wkv.tile([128, 4 * 1024], BF16, tag="wkv", name=f"wkv{i}")
        xt[i] = p_xt.tile([128, 4 * 512], BF16, tag="xt", name=f"xt{i}")
        nc.sync.dma_start(
            xt[i][:, :].rearrange("p (kt n) -> p kt n", n=512),
            dio[f"xT{i}"].rearrange("(kt p) n -> p kt n", p=128),
        )
        nc.sync.dma_start(
            wkv[i][:, :].rearrange("p (kt x) -> p kt x", x=1024),
            dio[f"Wqkv{i}"].rearrange("(kt p) x -> p kt x", p=128)[:, :, 512:1536],
        )
    for i in range(3):
        wo[i] = p_wo.tile([128, 4 * 512], BF16, tag="wo", name=f"wo{i}")
        nc.sync.dma_start(
            wo[i][:, :].rearrange("p (kt x) -> p kt x", x=512),
            dio[f"Wout{i}"].rearrange("(kt p) x -> p kt x", p=128),
        )

    # ---- persistent SBUF layouts ----
    # qT_all[128, 4*W]: block fh holds dims of head-pair (2fh, 2fh+1); within
    # a block the three modalities' queries sit side by side (i*n_c + q).
    qT_all = p_qkv.tile([128, 4 * W], BF16, tag="qT", name="qT_all")
    # kT_all[128, 4*1536]: block fh, then modality j * 512 + key.
    kT_all = p_qkv.tile([128, 4 * 1536], BF16, tag="kT", name="kT_all")
    # V_all[128, 12*520]: chunk ci = j*4+c of 128 keys; within: 8 heads x
    # (64 V-dims + ones column).
    V_all = p_qkv.tile([128, 12 * 520], BF16, tag="V", name="V_all")

    def proj_q(i, cc):
        ps = psD.tile([128, PSW], f32, tag="d", name=f"psq{i}_{cc}")
        for kt in range(4):
            mm(
                ps[:, 0:n_c],
                wq[i][:, kt * 512 + cc * 128 : kt * 512 + cc * 128 + 128],
                xqt[i][:, kt * n_c : (kt + 1) * n_c],
                kt == 0,
                kt == 3,
            )
        nc.vector.tensor_copy(
            qT_all[:, cc * W + i * n_c : cc * W + (i + 1) * n_c],
            ps[:, 0:n_c],
        )

    def proj_k(i, cc):
        ps = psD.tile([128, PSW], f32, tag="d", name=f"psk{i}_{cc}")
        for kt in range(4):
            mm(
                ps[:, 0:512],
                wkv[i][:, kt * 1024 + cc * 128 : kt * 1024 + cc * 128 + 128],
                xt[i][:, kt * 512 : (kt + 1) * 512],
                kt == 0,
                kt == 3,
            )
        nc.vector.tensor_copy(
            kT_all[:, cc * 1536 + i * 512 : cc * 1536 + (i + 1) * 512],
            ps[:, 0:512],
        )

    def proj_v(i, nch):
        ps = psD.tile([128, PSW], f32, tag="d", name=f"psv{i}_{nch}")
        for kt in range(4):
            mm(
                ps[:, 0:512],
                xt[i][:, kt * 512 + nch * 128 : kt * 512 + nch * 128 + 128],
                wkv[i][:, kt * 1024 + 512 : kt * 1024 + 1024],
                kt == 0,
                kt == 3,
            )
        ci = i * 4 + nch
        dst = V_all[:, ci * 520 : (ci + 1) * 520].rearrange(
            "p (h x) -> p h x", x=65
        )
        nc.vector.tensor_copy(
            dst[:, :, 0:64], ps[:, 0:512].rearrange("p (h x) -> p h x", x=64)
        )
        nc.vector.tensor_copy(
            dst[:, :, 64:65], ones8[:, :].rearrange("p (h x) -> p h x", x=1)
        )

    # ---- attention stream (software-pipelined, with proj fillers) ----
    AV_LAG = 2
    # Emission points (h, kc) -> projection units emitted just before that
    # dots call.  v(j, c) must precede av(h0, j*4+c) (emitted at kc j*4+c+2);
    # k(j, cc) before the first dots of fh block cc on modality j; q(*, cc)
    # before head pair cc starts.
    fillers = {
        (0, 1): [(proj_v, 0, 0)],
        (0, 2): [(proj_v, 0, 1)],
        (0, 3): [(proj_v, 0, 2), (proj_k, 1, 0)],
        (0, 4): [(proj_v, 0, 3)],
        (0, 5): [(proj_v, 1, 0)],
        (0, 6): [(proj_v, 1, 1), (proj_k, 2, 0)],
        (0, 7): [(proj_v, 1, 2)],
        (0, 8): [(proj_v, 1, 3)],
        (0, 9): [(proj_v, 2, 0)],
        (0, 10): [(proj_v, 2, 1)],
        (0, 11): [(proj_v, 2, 2)],
        (1, 0): [(proj_v, 2, 3)],
        (1, 2): [(proj_q, 0, 1)],
        (1, 4): [(proj_q, 1, 1)],
        (1, 6): [(proj_q, 2, 1)],
        (1, 8): [(proj_k, 0, 1)],
        (1, 10): [(proj_k, 1, 1)],
        (2, 0): [(proj_k, 2, 1)],
        (3, 0): [(proj_q, 0, 2)],
        (3, 2): [(proj_q, 1, 2)],
        (3, 4): [(proj_q, 2, 2)],
        (3, 6): [(proj_k, 0, 2)],
        (3, 8): [(proj_k, 1, 2)],
        (3, 10): [(proj_k, 2, 2)],
        (5, 0): [(proj_q, 0, 3)],
        (5, 2): [(proj_q, 1, 3)],
        (5, 4): [(proj_q, 2, 3)],
        (5, 6): [(proj_k, 0, 3)],
        (5, 8): [(proj_k, 1, 3)],
        (5, 10): [(proj_k, 2, 3)],
    }

    ps_d_t, ex_t, ps_o_t = {}, {}, {}

    def dots(h, kc):
        fh, po = h // 2, (h % 2) * 64
        j, c = kc // 4, kc % 4
        ps_d = psD.tile([128, PSW], f32, tag="d", name=f"psd{h}_{kc}")
        ps_d_t[(h, kc)] = ps_d
        base = fh * 1536 + j * 512 + c * 128
        kh = kT_all[po : po + 64, base : base + 128]
        for s in range(NS):
            mm(
                ps_d[:, s * 512 : s * 512 + WS],
                kh,
                qT_all[po : po + 64, fh * W + s * WS : fh * W + (s + 1) * WS],
                True,
                True,
            )
        ex = p_ex.tile([128, W], BF16, tag="ex", name=f"ex{h}_{kc}")
        ex_t[(h, kc)] = ex
        nc.scalar.activation(
            ex[:, :].rearrange("p (s x) -> p s x", x=WS),
            ps_d[:, :].rearrange("p (s x) -> p s x", x=512)[:, :, 0:WS],
            Exp,
            scale=SCALE,
        )

    def av(h, kc):
        if kc == 0:
            ps_o_t[h] = psO.tile([128, PSW], f32, tag="o", name=f"pso{h}")
        ps_o = ps_o_t[h]
        ci = kc  # chunk index == kc (j*4+c ordering matches V_all layout)
        vh = V_all[:, ci * 520 + h * 65 : ci * 520 + h * 65 + 65]
        ex = ex_t.pop((h, kc))
        for s in range(NS):
            mm(
                ps_o[0:65, s * 512 : s * 512 + WS],
                vh,
                ex[:, s * WS : (s + 1) * WS],
                kc == 0,
                kc == 11,
            )

    def norm(h):
        ps_o = ps_o_t.pop(h)
        # the custom-DVE reciprocal ignores the input partition offset, so
        # first move the denominator row (PSUM partition 64) to an SBUF
        # tile at partition 0.  The chain runs per bank-aligned piece so the
        # two halves pipeline (shorter pair-ready latency for the final head,
        # which gates the out-projection).
        den = p_den.tile([1, PSW], f32, tag="den", name=f"dn{h}")
        rden = p_den.tile([1, PSW], f32, tag="r", name=f"rd{h}")
        bc = p_den.tile([64, PSW], f32, tag="b", name=f"bc{h}")
        po2 = (h % 2) * 64
        dst = pair[h // 2][po2 : po2 + 64, :]
        for s in range(NS):
            sl = slice(s * 512, s * 512 + WS)
            nc.vector.tensor_copy(den[0:1, sl], ps_o[64:65, sl])
            nc.vector.reciprocal_approx_fast(
                out=rden[0:1, sl], in_=den[0:1, sl]
            )
            nc.gpsimd.partition_broadcast(bc[:, sl], rden[0:1, sl])
            nc.vector.tensor_mul(
                dst[:, s * WS : (s + 1) * WS],
                ps_o[0:64, sl],
                bc[:, sl],
            )

    pair = [
        p_pair.tile([128, W], BF16, tag=f"pair{p}", name=f"pair{p}")
        for p in range(4)
    ]

    for i in range(3):
        proj_q(i, 0)
    proj_k(0, 0)

    for h in range(HEADS):
        for kc in range(12):
            for fn, i, cc in fillers.get((h, kc), ()):
                fn(i, cc)
            dots(h, kc)
            if kc == 1 and h > 0:
                av(h - 1, 10)
                av(h - 1, 11)
                norm(h - 1)
            if kc >= AV_LAG:
                av(h, kc - AV_LAG)

    # ---- out-projection, two-phase, overlapped with the attention tail ----
    # Phase A (pair0-2 contributions) only needs heads 0-5, so it runs on
    # the PE while the Scalar engine finishes head 7's exps and the
    # DVE/Pool engines run the last norm chains; phase B adds pair3,
    # copies out (alternating Scalar/Vector, both idle by then), and DMAs.
    # Five packed claims (the two 128-row chunks of one modality per claim;
    # the leftover chunks of different modalities share a claim) keep the
    # claim count close to the 4 available PSUM ring slots.
    groups = []  # each: list of (i, s, nn, bank)
    tail_grp = []
    for i in range(3):
        chunks = [(s, min(128, n_c - s)) for s in range(0, n_c, 128)]
        for g in range(0, len(chunks) - len(chunks) % 2, 2):
            groups.append([(i, s, nn, gi) for gi, (s, nn) in
                           enumerate(chunks[g : g + 2])])
        if len(chunks) % 2:
            s, nn = chunks[-1]
            tail_grp.append((i, s, nn))
    for t in range(0, len(tail_grp), NS):
        groups.append([(i, s, nn, gi % NS) for gi, (i, s, nn) in
                       enumerate(tail_grp[t : t + NS])])
    if ps_bufs == 1:
        # single-buffered PSUM rings (NS == 3 fallback): the phase-A
        # interleave below would deadlock the in-order PE queue, so finish
        # the last head before any out-projection claim
        av(HEADS - 1, 10)
        av(HEADS - 1, 11)
        norm(HEADS - 1)
    claims = []
    for gidx, grp in enumerate(groups):
        pool = psD if gidx % 2 == 0 else psO
        ps = pool.tile([128, PSW], f32, tag="d" if pool is psD else "o",
                       name=f"psf{gidx}")
        claims.append(ps)
        for i, s, nn, bank in grp:
            for p in range(3):
                mm(
                    ps[0:nn, bank * 512 : bank * 512 + 512],
                    pair[p][:, i * n_c + s : i * n_c + s + nn],
                    wo[i][:, p * 512 : (p + 1) * 512],
                    p == 0,
                    False,
                )
        if gidx == 1 and ps_bufs == 2:
            av(HEADS - 1, 10)
            av(HEADS - 1, 11)
            norm(HEADS - 1)
    for gidx, grp in enumerate(groups):
        ps = claims[gidx]
        for i, s, nn, bank in grp:
            mm(
                ps[0:nn, bank * 512 : bank * 512 + 512],
                pair[3][:, i * n_c + s : i * n_c + s + nn],
                wo[i][:, 3 * 512 : 4 * 512],
                False,
                True,
            )
        ob = p_ob.tile([128, PSW], f32, tag="ob", name=f"ob{gidx}")
        for k, (i, s, nn, bank) in enumerate(grp):
            cp = nc.scalar.copy if (gidx + k) % 2 == 0 else nc.vector.tensor_copy
            cp(
                ob[0:nn, bank * 512 : bank * 512 + 512],
                ps[0:nn, bank * 512 : bank * 512 + 512],
            )
            nc.sync.dma_start(
                dio[f"out{i}"][s : s + nn, :],
                ob[0:nn, bank * 512 : bank * 512 + 512],
            )


def build(n_c=280, reps=1):
    nc = bacc.Bacc("TRN2", target_bir_lowering=False, debug=False)
    NS, _ = _splits(n_c)
    ps_bufs = 2 if NS == 2 else 1
    dio = {}
    for i in range(3):
        dio[f"xT{i}"] = nc.dram_tensor(f"xT{i}", [D, N], BF16, kind="ExternalInput").ap()
        dio[f"xqT{i}"] = nc.dram_tensor(
            f"xqT{i}", [D, n_c], BF16, kind="ExternalInput"
        ).ap()
        dio[f"Wqkv{i}"] = nc.dram_tensor(
            f"Wqkv{i}", [D, 3 * DI], BF16, kind="ExternalInput"
        ).ap()
        dio[f"Wout{i}"] = nc.dram_tensor(
            f"Wout{i}", [DI, D], BF16, kind="ExternalInput"
        ).ap()
        dio[f"out{i}"] = nc.dram_tensor(
            f"out{i}", [n_c, D], F32, kind="ExternalOutput"
        ).ap()
    with tile.TileContext(nc) as tc:
        with (
            tc.tile_pool(name="wq", bufs=3) as p_wq,
            tc.tile_pool(name="wkv", bufs=3) as p_wkv,
            tc.tile_pool(name="xt", bufs=3) as p_xt,
            tc.tile_pool(name="xqt", bufs=3) as p_xqt,
            tc.tile_pool(name="qkv", bufs=1) as p_qkv,
            tc.tile_pool(name="ex", bufs=4) as p_ex,
            tc.tile_pool(name="den", bufs=2) as p_den,
            tc.tile_pool(name="pair", bufs=1) as p_pair,
            tc.tile_pool(name="wo", bufs=3) as p_wo,
            tc.tile_pool(name="ob", bufs=2) as p_ob,
            tc.tile_pool(name="psD", bufs=ps_bufs, space="PSUM") as psD,
            tc.tile_pool(name="psO", bufs=ps_bufs, space="PSUM") as psO,
        ):
            pools = (p_wq, p_wkv, p_xt, p_xqt, p_qkv, p_ex, p_den, p_pair,
                     p_wo, p_ob, psD, psO)
            for _ in range(reps):
                _emit_body(nc, tc, dio, pools, n_c)
    nc.compile()
    return nc


_BUILD_CACHE = {}


def _get_built(n_c, reps):
    key = (n_c, reps)
    if key not in _BUILD_CACHE:
        _BUILD_CACHE[key] = build(n_c, reps)
    return _BUILD_CACHE[key]


def pick_n_c(inputs):
    """Smallest supported compacted-query count for these masks.

    Per (b, i) we need room for the unmasked queries plus one zero (dummy)
    row whose output serves every masked query of that (b, i)."""
    need = 0
    for i in range(3):
        m = np.asarray(inputs[f"m{i}"]).astype(bool)
        for b in range(B):
            n_u = int(m[b].sum())
            need = max(need, n_u + (1 if n_u < N else 0))
    n_c = max(256, -(-need // 8) * 8)
    if n_c > 336:  # NS=2 supports 3*n_c/2 <= 512
        for cand in (384, 448, 512):
            if need <= cand:
                return cand
        return 512
    return n_c


def make_in_maps(inputs, n_c=280):
    """Build per-core input dicts.  The q-projection input is compacted to
    the unmasked query rows (plus zero padding; the first padding row doubles
    as the masked-row output).  x / xq / Wqkv / Wout ship as bf16."""
    bf = ml_dtypes.bfloat16
    xs = [np.asarray(inputs[f"x{i}"], dtype=np.float32) for i in range(3)]
    ms = [np.asarray(inputs[f"m{i}"]).astype(bool) for i in range(3)]
    Wq = [np.asarray(inputs[f"Wqkv{i}"], np.float32).astype(bf) for i in range(3)]
    Wo = [np.asarray(inputs[f"Wout{i}"], np.float32).astype(bf) for i in range(3)]
    in_maps = []
    for b in range(B):
        m = {}
        for i in range(3):
            xb = xs[i][b]
            m[f"xT{i}"] = np.ascontiguousarray(xb.T).astype(bf)
            if n_c == N:
                xq = xb * ms[i][b][:, None]
            else:
                sel = np.flatnonzero(ms[i][b])
                xq = np.zeros((n_c, D), np.float32)
                xq[: len(sel)] = xb[sel]
            m[f"xqT{i}"] = np.ascontiguousarray(xq.T).astype(bf)
            m[f"Wqkv{i}"] = Wq[i]
            m[f"Wout{i}"] = Wo[i]
        in_maps.append(m)
    return in_maps


def scatter_outputs(results, inputs, n_c):
    ms = [np.asarray(inputs[f"m{i}"]).astype(bool) for i in range(3)]
    outs = []
    for i in range(3):
        full = np.empty((B, N, D), np.float32)
        for b in range(B):
            comp = np.asarray(results[b][f"out{i}"], np.float32)
            if n_c == N:
                full[b] = comp
            else:
                sel = np.flatnonzero(ms[i][b])
                full[b][sel] = comp[: len(sel)]
                if len(sel) < N:
                    full[b][~ms[i][b]] = comp[len(sel)]
        outs.append(full)
    return outs


def kernel(**inputs):
    n_c = pick_n_c(inputs)
    in_maps = make_in_maps(inputs, n_c)
    nc = _get_built(n_c=n_c, reps=1)
    res = bass_utils.run_bass_kernel_spmd(nc, in_maps, core_ids=list(range(N_CORES)))
    return tuple(scatter_outputs(res.results, inputs, n_c))


# revision 41
# speedup vs baseline: 1.0952x; 1.0140x over previous
"""Trainium2 Bass kernel for nn_JointModalityAttention.

3-modality joint attention, B=8, N=512, D=512, 8 heads x 64.
Sharding: data-parallel over batch -- each of the 8 NeuronCores handles one
batch element; the weights are replicated. No collectives.

Dataflow (per core, fully transpose-free on device):
  - Host passes x transposed (xT[k, n]) and a q-projection copy with masked
    query rows compacted out (xqT), both bf16 (as are the Wqkv weights) to
    halve input-DMA bytes.  Zeroed q rows -> dots == 0 -> exp(0) == 1 ->
    uniform attention, which is exactly what the reference's whole-row -1e9
    masking produces, so one zero-padding row serves every masked query.
  - Projections run in bf16 and write bf16 SBUF operand tiles: bf16 weights
    halve the per-matmul LDWEIGHTS time, which otherwise serializes between
    matmuls and dominates small-N attention matmuls.
  - The three modalities' queries are packed side by side (W = 3*n_c wide) so
    one dots matmul serves all three: dots^T[key_chunk, q] = kh.T @ qh_all,
    split into NS bank-aligned pieces of WS columns.
  - exp on the Scalar engine straight out of PSUM with the 1/sqrt(dh) scale
    folded in (no max-subtraction: dots are O(1)).  One exp instruction per
    (head, key-chunk) covers all three modalities; the Scalar engine is the
    pacing engine, so projections are emitted per-fh-block as PE filler
    exactly where the attention stream needs them, keeping the exp stream
    dense from ~15us on.
  - attn@V shares the V tile as lhsT across modalities: o^T[d, q_all] accum
    over 12 key chunks; a ones column in V makes row 64 the softmax
    denominator for every query.
  - normalize: copy the denominator row to partition 0 (custom-DVE ops
    ignore input partition offsets), reciprocal_approx_fast, gpsimd
    partition-broadcast, one DVE multiply into the head-pair tile that is
    exactly the lhsT layout the out-projection needs.
  - Emission is software-pipelined: dots runs 2 key-chunks ahead of attn@V
    (PSUM double-buffering), heads are processed fh-major so each fh block's
    k/q projections are produced one pair ahead of use.
"""

import sys

if "/opt/trn_rl_repo" not in sys.path:
    sys.path.insert(0, "/opt/trn_rl_repo")

import ml_dtypes
import numpy as np

import concourse.bass as bass  # noqa: F401  (import keeps bass registered)
import concourse.mybir as mybir
import concourse.tile as tile
from concourse import bacc, bass_utils

HEADS = 8
DH = 64
DI = HEADS * DH  # 512
B = 8
N = 512
D = 512
SCALE = DH ** -0.5
N_CORES = 8

F32 = mybir.dt.float32
BF16 = mybir.dt.bfloat16
F32R = mybir.dt.float32r


def tf32_round(a):
    """Round fp32 to the TF32-representable subset (10-bit mantissa, RNE)."""
    u = np.ascontiguousarray(a, np.float32).view(np.uint32).copy()
    lsb = (u >> 13) & 1
    u += 0x0FFF + lsb
    u &= 0xFFFFE000
    return u.view(np.float32)


def _splits(n_c):
    """(NS, WS): number of bank-aligned pieces the merged q width 3*n_c is
    split into, and the piece width (<=512 per PSUM bank / moving max)."""
    W = 3 * n_c
    ns = 2 if W <= 1024 else 3
    assert W % ns == 0 and W // ns <= 512
    return ns, W // ns


def _emit_body(nc, tc, dio, pools, n_c):
    f32 = F32
    Exp = mybir.ActivationFunctionType.Exp
    (p_wq, p_wkv, p_xt, p_xqt, p_qkv, p_ex, p_den, p_pair, p_wo, p_ob,
     psD, psO) = pools
    NS, WS = _splits(n_c)
    W = 3 * n_c
    PSW = NS * 512
    ps_bufs = 2 if NS == 2 else 1

    def mm(out, lhsT, rhs, start, stop):
        return nc.tensor.matmul(out, lhsT, rhs, start=start, stop=stop)

    ones8 = p_den.tile([128, 8], f32, tag="ones8", name="ones8")
    nc.vector.memset(ones8[:, :], 1.0)

    # ---- input DMA: one instruction per tile.  The SP sequencer spends
    # ~6.5us on its start-of-kernel preamble before it can issue anything,
    # so the startup-critical transfers go out on the Activation and DVE
    # sequencers, which are ready at ~0.2us (both idle until the exp stream
    # / first copies anyway).
    wq, wkv, xt, xqt, wo = {}, {}, {}, {}, {}
    for i in range(3):
        wq[i] = p_wq.tile([128, 4 * 512], BF16, tag="wq", name=f"wq{i}")
        xqt[i] = p_xqt.tile([128, 4 * n_c], BF16, tag="xqt", name=f"xqt{i}")
        nc.sync.dma_start(
            wq[i][:, :].rearrange("p (kt x) -> p kt x", x=512),
            dio[f"Wqkv{i}"].rearrange("(kt p) x -> p kt x", p=128)[:, :, 0:512],
        )
        nc.sync.dma_start(
            xqt[i][:, :].rearrange("p (kt n) -> p kt n", n=n_c),
            dio[f"xqT{i}"].rearrange("(kt p) n -> p kt n", p=128),
        )
    for i in range(3):
        wkv[i] = p_wkv.tile([128, 4 * 1024], BF16, tag="wkv", name=f"wkv{i}")
        xt[i] = p_xt.tile([128, 4 * 512], BF16, tag="xt", name=f"xt{i}")
        nc.sync.dma_start(
            xt[i][:, :].rearrange("p (kt n) -> p kt n", n=512),
            dio[f"xT{i}"].rearrange("(kt p) n -> p kt n", p=128),
        )
        nc.sync.dma_start(
            wkv[i][:, :].rearrange("p (kt x) -> p kt x", x=1024),
            dio[f"Wqkv{i}"].rearrange("(kt p) x -> p kt x", p=128)[:, :, 512:1536],
        )
    for i in range(3):
        wo[i] = p_wo.tile([128, 4 * 512], BF16, tag="wo", name=f"wo{i}")
        nc.sync.dma_start(
            wo[i][:, :].rearrange("p (kt x) -> p kt x", x=512),
            dio[f"Wout{i}"].rearrange("(kt p) x -> p kt x", p=128),
        )

    # ---- persistent SBUF layouts ----
    # qT_all[128, 4*W]: block fh holds dims of head-pair (2fh, 2fh+1); within
    # a block the three modalities' queries sit side by side (i*n_c + q).
    qT_all = p_qkv.tile([128, 4 * W], BF16, tag="qT", name="qT_all")
    # kT_all[128, 4*1536]: block fh, then modality j * 512 + key.
    kT_all = p_qkv.tile([128, 4 * 1536], BF16, tag="kT", name="kT_all")
    # V_all[128, 12*520]: chunk ci = j*4+c of 128 keys; within: 8 heads x
    # (64 V-dims + ones column).
    V_all = p_qkv.tile([128, 12 * 520], BF16, tag="V", name="V_all")

    def proj_q(i, cc):
        ps = psD.tile([128, PSW], f32, tag="d", name=f"psq{i}_{cc}")
        for kt in range(4):
            mm(
                ps[:, 0:n_c],
                wq[i][:, kt * 512 + cc * 128 : kt * 512 + cc * 128 + 128],
                xqt[i][:, kt * n_c : (kt + 1) * n_c],
                kt == 0,
                kt == 3,
            )
        nc.vector.tensor_copy(
            qT_all[:, cc * W + i * n_c : cc * W + (i + 1) * n_c],
            ps[:, 0:n_c],
        )

    def proj_k(i, cc):
        ps = psD.tile([128, PSW], f32, tag="d", name=f"psk{i}_{cc}")
        for kt in range(4):
            mm(
                ps[:, 0:512],
                wkv[i][:, kt * 1024 + cc * 128 : kt * 1024 + cc * 128 + 128],
                xt[i][:, kt * 512 : (kt + 1) * 512],
                kt == 0,
                kt == 3,
            )
        nc.vector.tensor_copy(
            kT_all[:, cc * 1536 + i * 512 : cc * 1536 + (i + 1) * 512],
            ps[:, 0:512],
        )

    def proj_v(i, nch):
        ps = psD.tile([128, PSW], f32, tag="d", name=f"psv{i}_{nch}")
        for kt in range(4):
            mm(
                ps[:, 0:512],
                xt[i][:, kt * 512 + nch * 128 : kt * 512 + nch * 128 + 128],
                wkv[i][:, kt * 1024 + 512 : kt * 1024 + 1024],
                kt == 0,
                kt == 3,
            )
        ci = i * 4 + nch
        dst = V_all[:, ci * 520 : (ci + 1) * 520].rearrange(
            "p (h x) -> p h x", x=65
        )
        nc.vector.tensor_copy(
            dst[:, :, 0:64], ps[:, 0:512].rearrange("p (h x) -> p h x", x=64)
        )
        nc.vector.tensor_copy(
            dst[:, :, 64:65], ones8[:, :].rearrange("p (h x) -> p h x", x=1)
        )

    # ---- attention stream (software-pipelined, with proj fillers) ----
    AV_LAG = 2
    # Emission points (h, kc) -> projection units emitted just before that
    # dots call.  v(j, c) must precede av(h0, j*4+c) (emitted at kc j*4+c+2);
    # k(j, cc) before the first dots of fh block cc on modality j; q(*, cc)
    # before head pair cc starts.
    fillers = {
        (0, 1): [(proj_v, 0, 0)],
        (0, 2): [(proj_v, 0, 1)],
        (0, 3): [(proj_v, 0, 2), (proj_k, 1, 0)],
        (0, 4): [(proj_v, 0, 3)],
        (0, 5): [(proj_v, 1, 0)],
        (0, 6): [(proj_v, 1, 1), (proj_k, 2, 0)],
        (0, 7): [(proj_v, 1, 2)],
        (0, 8): [(proj_v, 1, 3)],
        (0, 9): [(proj_v, 2, 0)],
        (0, 10): [(proj_v, 2, 1)],
        (0, 11): [(proj_v, 2, 2)],
        (1, 0): [(proj_v, 2, 3)],
        (1, 2): [(proj_q, 0, 1)],
        (1, 4): [(proj_q, 1, 1)],
        (1, 6): [(proj_q, 2, 1)],
        (1, 8): [(proj_k, 0, 1)],
        (1, 10): [(proj_k, 1, 1)],
        (2, 0): [(proj_k, 2, 1)],
        (3, 0): [(proj_q, 0, 2)],
        (3, 2): [(proj_q, 1, 2)],
        (3, 4): [(proj_q, 2, 2)],
        (3, 6): [(proj_k, 0, 2)],
        (3, 8): [(proj_k, 1, 2)],
        (3, 10): [(proj_k, 2, 2)],
        (5, 0): [(proj_q, 0, 3)],
        (5, 2): [(proj_q, 1, 3)],
        (5, 4): [(proj_q, 2, 3)],
        (5, 6): [(proj_k, 0, 3)],
        (5, 8): [(proj_k, 1, 3)],
        (5, 10): [(proj_k, 2, 3)],
    }

    ps_d_t, ex_t, ps_o_t = {}, {}, {}

    def dots(h, kc):
        fh, po = h // 2, (h % 2) * 64
        j, c = kc // 4, kc % 4
        ps_d = psD.tile([128, PSW], f32, tag="d", name=f"psd{h}_{kc}")
        ps_d_t[(h, kc)] = ps_d
        base = fh * 1536 + j * 512 + c * 128
        kh = kT_all[po : po + 64, base : base + 128]
        for s in range(NS):
            mm(
                ps_d[:, s * 512 : s * 512 + WS],
                kh,
                qT_all[po : po + 64, fh * W + s * WS : fh * W + (s + 1) * WS],
                True,
                True,
            )
        ex = p_ex.tile([128, W], BF16, tag="ex", name=f"ex{h}_{kc}")
        ex_t[(h, kc)] = ex
        nc.scalar.activation(
            ex[:, :].rearrange("p (s x) -> p s x", x=WS),
            ps_d[:, :].rearrange("p (s x) -> p s x", x=512)[:, :, 0:WS],
            Exp,
            scale=SCALE,
        )

    def av(h, kc):
        if kc == 0:
            ps_o_t[h] = psO.tile([128, PSW], f32, tag="o", name=f"pso{h}")
        ps_o = ps_o_t[h]
        ci = kc  # chunk index == kc (j*4+c ordering matches V_all layout)
        vh = V_all[:, ci * 520 + h * 65 : ci * 520 + h * 65 + 65]
        ex = ex_t.pop((h, kc))
        for s in range(NS):
            mm(
                ps_o[0:65, s * 512 : s * 512 + WS],
                vh,
                ex[:, s * WS : (s + 1) * WS],
                kc == 0,
                kc == 11,
            )

    def norm(h):
        ps_o = ps_o_t.pop(h)
        # the custom-DVE reciprocal ignores the input partition offset, so
        # first move the denominator row (PSUM partition 64) to an SBUF
        # tile at partition 0.  The chain runs per bank-aligned piece so the
        # two halves pipeline (shorter pair-ready latency for the final head,
        # which gates the out-projection).
        den = p_den.tile([1, PSW], f32, tag="den", name=f"dn{h}")
        rden = p_den.tile([1, PSW], f32, tag="r", name=f"rd{h}")
        bc = p_den.tile([64, PSW], f32, tag="b", name=f"bc{h}")
        po2 = (h % 2) * 64
        dst = pair[h // 2][po2 : po2 + 64, :]
        for s in range(NS):
            sl = slice(s * 512, s * 512 + WS)
            nc.vector.tensor_copy(den[0:1, sl], ps_o[64:65, sl])
            nc.vector.reciprocal_approx_fast(
                out=rden[0:1, sl], in_=den[0:1, sl]
            )
            nc.gpsimd.partition_broadcast(bc[:, sl], rden[0:1, sl])
            nc.vector.tensor_mul(
                dst[:, s * WS : (s + 1) * WS],
                ps_o[0:64, sl],
                bc[:, sl],
            )

    pair = [
        p_pair.tile([128, W], BF16, tag=f"pair{p}", name=f"pair{p}")
        for p in range(4)
    ]

    for i in range(3):
        proj_q(i, 0)
    proj_k(0, 0)

    # Key-chunks are processed in pairs: both dots before both attn@V calls
    # reduces PE weight-switches (kh->kh'->vh->vh') from 2 to 1.5 per chunk;
    # each switch exposes ~90ns of LDWEIGHTS on the PE-bound stream.
    for h in range(HEADS):
        for kc in range(0, 12, 2):
            for pos in (kc, kc + 1):
                for fn, i, cc in fillers.get((h, pos), ()):
                    fn(i, cc)
            dots(h, kc)
            dots(h, kc + 1)
            if kc == 0 and h > 0:
                av(h - 1, 10)
                av(h - 1, 11)
                norm(h - 1)
            if kc >= AV_LAG:
                av(h, kc - AV_LAG)
                av(h, kc - AV_LAG + 1)

    # ---- out-projection, two-phase, overlapped with the attention tail ----
    # Phase A (pair0-2 contributions) only needs heads 0-5, so it runs on
    # the PE while the Scalar engine finishes head 7's exps and the
    # DVE/Pool engines run the last norm chains; phase B adds pair3,
    # copies out (alternating Scalar/Vector, both idle by then), and DMAs.
    # Five packed claims (the two 128-row chunks of one modality per claim;
    # the leftover chunks of different modalities share a claim) keep the
    # claim count close to the 4 available PSUM ring slots.
    groups = []  # each: list of (i, s, nn, bank)
    tail_grp = []
    for i in range(3):
        chunks = [(s, min(128, n_c - s)) for s in range(0, n_c, 128)]
        for g in range(0, len(chunks) - len(chunks) % 2, 2):
            groups.append([(i, s, nn, gi) for gi, (s, nn) in
                           enumerate(chunks[g : g + 2])])
        if len(chunks) % 2:
            s, nn = chunks[-1]
            tail_grp.append((i, s, nn))
    for t in range(0, len(tail_grp), NS):
        groups.append([(i, s, nn, gi % NS) for gi, (i, s, nn) in
                       enumerate(tail_grp[t : t + NS])])
    if ps_bufs == 1:
        # single-buffered PSUM rings (NS == 3 fallback): the phase-A
        # interleave below would deadlock the in-order PE queue, so finish
        # the last head before any out-projection claim
        av(HEADS - 1, 10)
        av(HEADS - 1, 11)
        norm(HEADS - 1)
    claims = []
    for gidx, grp in enumerate(groups):
        pool = psD if gidx % 2 == 0 else psO
        ps = pool.tile([128, PSW], f32, tag="d" if pool is psD else "o",
                       name=f"psf{gidx}")
        claims.append(ps)
        for i, s, nn, bank in grp:
            for p in range(3):
                mm(
                    ps[0:nn, bank * 512 : bank * 512 + 512],
                    pair[p][:, i * n_c + s : i * n_c + s + nn],
                    wo[i][:, p * 512 : (p + 1) * 512],
                    p == 0,
                    False,
                )
        if gidx == 1 and ps_bufs == 2:
            av(HEADS - 1, 10)
            av(HEADS - 1, 11)
            norm(HEADS - 1)
    for gidx, grp in enumerate(groups):
        ps = claims[gidx]
        for i, s, nn, bank in grp:
            mm(
                ps[0:nn, bank * 512 : bank * 512 + 512],
                pair[3][:, i * n_c + s : i * n_c + s + nn],
                wo[i][:, 3 * 512 : 4 * 512],
                False,
                True,
            )
        ob = p_ob.tile([128, PSW], f32, tag="ob", name=f"ob{gidx}")
        for k, (i, s, nn, bank) in enumerate(grp):
            cp = nc.scalar.copy if (gidx + k) % 2 == 0 else nc.vector.tensor_copy
            cp(
                ob[0:nn, bank * 512 : bank * 512 + 512],
                ps[0:nn, bank * 512 : bank * 512 + 512],
            )
            nc.sync.dma_start(
                dio[f"out{i}"][s : s + nn, :],
                ob[0:nn, bank * 512 : bank * 512 + 512],
            )


def build(n_c=280, reps=1):
    nc = bacc.Bacc("TRN2", target_bir_lowering=False, debug=False)
    NS, _ = _splits(n_c)
    ps_bufs = 2 if NS == 2 else 1
    dio = {}
    for i in range(3):
        dio[f"xT{i}"] = nc.dram_tensor(f"xT{i}", [D, N], BF16, kind="ExternalInput").ap()
        dio[f"xqT{i}"] = nc.dram_tensor(
            f"xqT{i}", [D, n_c], BF16, kind="ExternalInput"
        ).ap()
        dio[f"Wqkv{i}"] = nc.dram_tensor(
            f"Wqkv{i}", [D, 3 * DI], BF16, kind="ExternalInput"
        ).ap()
        dio[f"Wout{i}"] = nc.dram_tensor(
            f"Wout{i}", [DI, D], BF16, kind="ExternalInput"
        ).ap()
        dio[f"out{i}"] = nc.dram_tensor(
            f"out{i}", [n_c, D], F32, kind="ExternalOutput"
        ).ap()
    with tile.TileContext(nc) as tc:
        with (
            tc.tile_pool(name="wq", bufs=3) as p_wq,
            tc.tile_pool(name="wkv", bufs=3) as p_wkv,
            tc.tile_pool(name="xt", bufs=3) as p_xt,
            tc.tile_pool(name="xqt", bufs=3) as p_xqt,
            tc.tile_pool(name="qkv", bufs=1) as p_qkv,
            tc.tile_pool(name="ex", bufs=4) as p_ex,
            tc.tile_pool(name="den", bufs=2) as p_den,
            tc.tile_pool(name="pair", bufs=1) as p_pair,
            tc.tile_pool(name="wo", bufs=3) as p_wo,
            tc.tile_pool(name="ob", bufs=2) as p_ob,
            tc.tile_pool(name="psD", bufs=ps_bufs, space="PSUM") as psD,
            tc.tile_pool(name="psO", bufs=ps_bufs, space="PSUM") as psO,
        ):
            pools = (p_wq, p_wkv, p_xt, p_xqt, p_qkv, p_ex, p_den, p_pair,
                     p_wo, p_ob, psD, psO)
            for _ in range(reps):
                _emit_body(nc, tc, dio, pools, n_c)
    nc.compile()
    return nc


_BUILD_CACHE = {}


def _get_built(n_c, reps):
    key = (n_c, reps)
    if key not in _BUILD_CACHE:
        _BUILD_CACHE[key] = build(n_c, reps)
    return _BUILD_CACHE[key]


def pick_n_c(inputs):
    """Smallest supported compacted-query count for these masks.

    Per (b, i) we need room for the unmasked queries plus one zero (dummy)
    row whose output serves every masked query of that (b, i)."""
    need = 0
    for i in range(3):
        m = np.asarray(inputs[f"m{i}"]).astype(bool)
        for b in range(B):
            n_u = int(m[b].sum())
            need = max(need, n_u + (1 if n_u < N else 0))
    n_c = max(256, -(-need // 8) * 8)
    if n_c > 336:  # NS=2 supports 3*n_c/2 <= 512
        for cand in (384, 448, 512):
            if need <= cand:
                return cand
        return 512
    return n_c


def make_in_maps(inputs, n_c=280):
    """Build per-core input dicts.  The q-projection input is compacted to
    the unmasked query rows (plus zero padding; the first padding row doubles
    as the masked-row output).  x / xq / Wqkv / Wout ship as bf16."""
    bf = ml_dtypes.bfloat16
    xs = [np.asarray(inputs[f"x{i}"], dtype=np.float32) for i in range(3)]
    ms = [np.asarray(inputs[f"m{i}"]).astype(bool) for i in range(3)]
    Wq = [np.asarray(inputs[f"Wqkv{i}"], np.float32).astype(bf) for i in range(3)]
    Wo = [np.asarray(inputs[f"Wout{i}"], np.float32).astype(bf) for i in range(3)]
    in_maps = []
    for b in range(B):
        m = {}
        for i in range(3):
            xb = xs[i][b]
            m[f"xT{i}"] = np.ascontiguousarray(xb.T).astype(bf)
            if n_c == N:
                xq = xb * ms[i][b][:, None]
            else:
                sel = np.flatnonzero(ms[i][b])
                xq = np.zeros((n_c, D), np.float32)
                xq[: len(sel)] = xb[sel]
            m[f"xqT{i}"] = np.ascontiguousarray(xq.T).astype(bf)
            m[f"Wqkv{i}"] = Wq[i]
            m[f"Wout{i}"] = Wo[i]
        in_maps.append(m)
    return in_maps


def scatter_outputs(results, inputs, n_c):
    ms = [np.asarray(inputs[f"m{i}"]).astype(bool) for i in range(3)]
    outs = []
    for i in range(3):
        full = np.empty((B, N, D), np.float32)
        for b in range(B):
            comp = np.asarray(results[b][f"out{i}"], np.float32)
            if n_c == N:
                full[b] = comp
            else:
                sel = np.flatnonzero(ms[i][b])
                full[b][sel] = comp[: len(sel)]
                if len(sel) < N:
                    full[b][~ms[i][b]] = comp[len(sel)]
        outs.append(full)
    return outs


def kernel(**inputs):
    n_c = pick_n_c(inputs)
    in_maps = make_in_maps(inputs, n_c)
    nc = _get_built(n_c=n_c, reps=1)
    res = bass_utils.run_bass_kernel_spmd(nc, in_maps, core_ids=list(range(N_CORES)))
    return tuple(scatter_outputs(res.results, inputs, n_c))
